# revision 6
# baseline (speedup 1.0000x reference)
"""Trainium2 Bass kernel for nn_PointWiseGlobalFusion.

Reference computation (B=2, N=5, C=64, H=W=256, G=4 groups, nf=64):
  emb1 = grouped_conv3x3(x, w1, b1); emb2 = grouped_conv3x3(x, w2, b2)
  cor[b,n,m,g,h,w] = sum_c emb1[b,n,g,c,h,w] * emb2[b,m,g,c,h,w]
  att  = softmax_m(cor)
  nl[b,n,g,c,h,w] = sum_m att[b,n,m,g,h,w] * x[b,m,g,c,h,w]
  pool = broadcast(max_n(x))
  out  = concat([x, nl, pool], channel axis)  -> [2, 5, 192, 256, 256]

Sharding: 8 cores = (b in {0,1}) x (4 H-quarters of 64 rows).  Each core gets
a zero-padded x slice [65, 5, 66, 258] (channel-major; row 64 = constant 1.0
used to fold the conv bias into the matmul) and produces [5, 192, 64, 256].
"""

import os
import sys

import numpy as np

if "/opt/trn_rl_repo" not in sys.path:
    sys.path.insert(0, "/opt/trn_rl_repo")

B, N, C, H, W = 2, 5, 64, 256, 256
G = 4
CG = C // G  # 16 channels per group
HQ = H // 4  # 64 rows per core
NPAIR = N * N

ROWS_PER_STRIP = 4
N_STRIPS = HQ // ROWS_PER_STRIP
ROWS_PER_CHUNK = 2  # matmul free dim = 2*256 = 512
CHUNK_PX = ROWS_PER_CHUNK * W

_CACHE = {}


def _build_masks():
    """Host-built constant matrices for the PE helper matmuls (fp32)."""
    # cor reduce: prod rows c (0..63) -> cor row 4*(5n+m) + c//16
    red = np.zeros((NPAIR, C, 4 * NPAIR), dtype=np.float32)
    for k in range(NPAIR):
        for c in range(C):
            red[k, c, 4 * k + c // CG] = 1.0
    # softmax denominator: cor row 4*(5n+m)+g -> col 4*n+g
    summ = np.zeros((4 * NPAIR, 4 * N), dtype=np.float32)
    for n in range(N):
        for m in range(N):
            for g in range(G):
                summ[4 * (5 * n + m) + g, 4 * n + g] = 1.0
    # recip broadcast: row 4*n+g -> cols 4*(5n+m)+g for all m
    rbm = np.zeros((4 * N, 4 * NPAIR), dtype=np.float32)
    for n in range(N):
        for m in range(N):
            for g in range(G):
                rbm[4 * n + g, 4 * (5 * n + m) + g] = 1.0
    # att broadcast over c: for pair k, row 4k+g -> cols g*16+cc
    attb = np.zeros((NPAIR, 4 * NPAIR, C), dtype=np.float32)
    for k in range(NPAIR):
        for g in range(G):
            attb[k, 4 * k + g, g * CG : (g + 1) * CG] = 1.0
    return red, summ, rbm, attb


def _build_conv_weights(w1, b1, w2, b2):
    """w912[tap, 65, 128]: lhsT per tap; row 64 of tap 4 carries the biases."""
    w912 = np.zeros((9, C + 1, 2 * C), dtype=np.float32)
    for t in range(9):
        dy, dx = t // 3, t % 3
        for o in range(C):
            g = o // CG
            w912[t, g * CG : (g + 1) * CG, o] = w1[o, :, dy, dx]
            w912[t, g * CG : (g + 1) * CG, C + o] = w2[o, :, dy, dx]
    w912[4, C, 0:C] = b1
    w912[4, C, C : 2 * C] = b2
    return w912


def _build_bass():
    import concourse.bacc as bacc
    import concourse.mybir as mybir
    from concourse import tile

    DT = mybir.dt.float32
    nc = bacc.Bacc("TRN2", target_bir_lowering=False)

    x_in = nc.dram_tensor("x", [C + 1, N, HQ + 2, W + 2], DT, kind="ExternalInput")
    w_in = nc.dram_tensor("w", [9, C + 1, 2 * C], DT, kind="ExternalInput")
    red_in = nc.dram_tensor("red", [NPAIR, C, 4 * NPAIR], DT, kind="ExternalInput")
    summ_in = nc.dram_tensor("summ", [4 * NPAIR, 4 * N], DT, kind="ExternalInput")
    rbm_in = nc.dram_tensor("rbm", [4 * N, 4 * NPAIR], DT, kind="ExternalInput")
    attb_in = nc.dram_tensor("attb", [NPAIR, 4 * NPAIR, C], DT, kind="ExternalInput")
    out = nc.dram_tensor("out", [N, 3 * C, HQ, W], DT, kind="ExternalOutput")

    EXP = mybir.ActivationFunctionType.Exp

    with tile.TileContext(nc) as tc:
        with (
            tc.tile_pool(name="const", bufs=1) as cp,
            tc.tile_pool(name="xs", bufs=2) as xp,
            tc.tile_pool(name="work", bufs=2) as wp,
            tc.tile_pool(name="small", bufs=4) as sp,
            tc.tile_pool(name="psA", bufs=2, space="PSUM") as psA,
            tc.tile_pool(name="psB", bufs=2, space="PSUM") as psB,
            tc.tile_pool(name="psC", bufs=1, space="PSUM") as psC,
            tc.tile_pool(name="psD", bufs=2, space="PSUM") as psD,
        ):
            wt = cp.tile([C + 1, 9, 2 * C], DT)
            nc.sync.dma_start(wt[:], w_in[:].transpose([1, 0, 2]))
            red_t = cp.tile([C, NPAIR, 4 * NPAIR], DT)
            nc.sync.dma_start(red_t[:], red_in[:].transpose([1, 0, 2]))
            summ_t = cp.tile([4 * NPAIR, 4 * N], DT)
            nc.sync.dma_start(summ_t[:], summ_in[:])
            rbm_t = cp.tile([4 * N, 4 * NPAIR], DT)
            nc.sync.dma_start(rbm_t[:], rbm_in[:])
            attb_t = cp.tile([4 * NPAIR, NPAIR, C], DT)
            nc.sync.dma_start(attb_t[:], attb_in[:].transpose([1, 0, 2]))

            for s in range(N_STRIPS):
                xs = xp.tile([C + 1, N, ROWS_PER_STRIP + 2, W + 2], DT)
                nc.sync.dma_start(
                    xs[:],
                    x_in[:, :, s * ROWS_PER_STRIP : s * ROWS_PER_STRIP + ROWS_PER_STRIP + 2, :],
                )

                # frame-max pool over the strip's center rows
                pool_t = wp.tile([C, ROWS_PER_STRIP, W], DT, tag="pool")
                nc.vector.tensor_max(
                    pool_t[:],
                    xs[0:C, 0, 1 : 1 + ROWS_PER_STRIP, 1 : 1 + W],
                    xs[0:C, 1, 1 : 1 + ROWS_PER_STRIP, 1 : 1 + W],
                )
                for f in range(2, N):
                    nc.vector.tensor_max(
                        pool_t[:], pool_t[:], xs[0:C, f, 1 : 1 + ROWS_PER_STRIP, 1 : 1 + W]
                    )

                for rc in range(ROWS_PER_STRIP // ROWS_PER_CHUNK):
                    # ---- grouped 3x3 convs (both convs fused, bias folded) ----
                    E1 = wp.tile([C, N, CHUNK_PX], DT, tag="E1")
                    E2 = wp.tile([C, N, CHUNK_PX], DT, tag="E2")
                    for f in range(N):
                        pe = psA.tile([2 * C, CHUNK_PX], DT)
                        for t in range(9):
                            dy, dx = t // 3, t % 3
                            rhs = xs[
                                0 : C + 1,
                                f,
                                rc * ROWS_PER_CHUNK + dy : rc * ROWS_PER_CHUNK + dy + ROWS_PER_CHUNK,
                                dx : dx + W,
                            ]
                            nc.tensor.matmul(
                                pe[:], wt[:, t, :], rhs, start=(t == 0), stop=(t == 8)
                            )
                        nc.scalar.copy(E1[:, f, :], pe[0:C, :])
                        nc.scalar.copy(E2[:, f, :], pe[C : 2 * C, :])

                    # ---- correlation: cor[4*(5n+m)+g] = sum_c e1[n,c]*e2[m,c] ----
                    pc = psB.tile([4 * NPAIR, CHUNK_PX], DT)
                    for n in range(N):
                        for m in range(N):
                            k = 5 * n + m
                            pr = sp.tile([C, CHUNK_PX], DT, tag="pr")
                            nc.vector.tensor_mul(pr[:], E1[:, n, :], E2[:, m, :])
                            nc.tensor.matmul(
                                pc[:], red_t[:, k, :], pr[:],
                                start=(k == 0), stop=(k == NPAIR - 1),
                            )

                    # ---- softmax over m ----
                    exps = wp.tile([4 * NPAIR, CHUNK_PX], DT, tag="exps")
                    nc.scalar.activation(exps[:], pc[:], EXP)
                    ps_s = psC.tile([4 * N, CHUNK_PX], DT, tag="s")
                    nc.tensor.matmul(ps_s[:], summ_t[:], exps[:], start=True, stop=True)
                    rec = sp.tile([4 * N, CHUNK_PX], DT, tag="rec")
                    nc.vector.reciprocal(rec[:], ps_s[:])
                    ps_rb = psC.tile([4 * NPAIR, CHUNK_PX], DT, tag="rb")
                    nc.tensor.matmul(ps_rb[:], rbm_t[:], rec[:], start=True, stop=True)
                    att = wp.tile([4 * NPAIR, CHUNK_PX], DT, tag="att")
                    nc.vector.tensor_mul(att[:], exps[:], ps_rb[:])

                    # ---- weighted sum over frames ----
                    nl = wp.tile([C, N, CHUNK_PX], DT, tag="nl")
                    for n in range(N):
                        for m in range(N):
                            k = 5 * n + m
                            pa = psD.tile([C, CHUNK_PX], DT)
                            nc.tensor.matmul(pa[:], attb_t[:, k, :], att[:], start=True, stop=True)
                            xm = xs[
                                0:C,
                                m,
                                rc * ROWS_PER_CHUNK + 1 : rc * ROWS_PER_CHUNK + 1 + ROWS_PER_CHUNK,
                                1 : 1 + W,
                            ]
                            if m == 0:
                                nc.vector.tensor_mul(nl[:, n, :], pa[:], xm)
                            else:
                                tmp = sp.tile([C, CHUNK_PX], DT, tag="wtmp")
                                nc.vector.tensor_mul(tmp[:], pa[:], xm)
                                nc.vector.tensor_add(nl[:, n, :], nl[:, n, :], tmp[:])

                    # ---- output DMAs for these 2 rows ----
                    r0 = s * ROWS_PER_STRIP + rc * ROWS_PER_CHUNK
                    for f in range(N):
                        nc.sync.dma_start(
                            out[f, 0:C, r0 : r0 + ROWS_PER_CHUNK, :],
                            xs[
                                0:C,
                                f,
                                rc * ROWS_PER_CHUNK + 1 : rc * ROWS_PER_CHUNK + 1 + ROWS_PER_CHUNK,
                                1 : 1 + W,
                            ],
                        )
                        nc.sync.dma_start(
                            out[f, C : 2 * C, r0 : r0 + ROWS_PER_CHUNK, :], nl[:, f, :]
                        )
                        nc.sync.dma_start(
                            out[f, 2 * C : 3 * C, r0 : r0 + ROWS_PER_CHUNK, :],
                            pool_t[:, rc * ROWS_PER_CHUNK : rc * ROWS_PER_CHUNK + ROWS_PER_CHUNK, :],
                        )

    nc.compile()
    return nc


def _get_nc():
    if "nc" not in _CACHE:
        _CACHE["nc"] = _build_bass()
    return _CACHE["nc"]


def _shard_x(x):
    """x: [2,5,64,256,256] f32 -> list of 8 per-core arrays [65,5,66,258]."""
    shards = []
    xpad = np.pad(x, ((0, 0), (0, 0), (0, 0), (1, 1), (1, 1)))  # pad H and W
    for core in range(8):
        b, q = divmod(core, 4)
        sl = xpad[b, :, :, q * HQ : q * HQ + HQ + 2, :]  # [5, 64, 66, 258]
        sl = np.ascontiguousarray(sl.transpose(1, 0, 2, 3))  # [64, 5, 66, 258]
        ones = np.ones((1,) + sl.shape[1:], dtype=np.float32)
        shards.append(np.concatenate([sl, ones], axis=0))
    return shards


def _ensure_ntff_hook():
    """Provide antenv.axon_hooks if the image lacks it, so trace=True works."""
    import types

    try:
        from antenv.axon_hooks import get_axon_ntff_profile_hook  # noqa: F401

        return
    except ImportError:
        pass
    import antenv

    mod = types.ModuleType("antenv.axon_hooks")
    _state = {"hook": None}
    mod.set_axon_ntff_profile_hook = lambda h: _state.__setitem__("hook", h)
    mod.get_axon_ntff_profile_hook = lambda: _state["hook"]
    sys.modules["antenv.axon_hooks"] = mod
    antenv.axon_hooks = mod
    try:
        from trn_agent_boot.trn_boot import _ntff_profile_via_ctypes

        mod.set_axon_ntff_profile_hook(
            _ntff_profile_via_ctypes("/opt/axon/libaxon_pjrt.so")
        )
    except Exception as e:  # degrade to no tracing
        print(f"ntff hook setup failed: {e}", file=sys.stderr)


def kernel(x, w1, b1, w2, b2):
    from concourse.bass_utils import run_bass_kernel_spmd

    x = np.asarray(x, dtype=np.float32)
    w1 = np.asarray(w1, dtype=np.float32)
    b1 = np.asarray(b1, dtype=np.float32)
    w2 = np.asarray(w2, dtype=np.float32)
    b2 = np.asarray(b2, dtype=np.float32)

    nc = _get_nc()
    w912 = _build_conv_weights(w1, b1, w2, b2)
    red, summ, rbm, attb = _build_masks()
    shards = _shard_x(x)
    in_maps = [
        {"x": shards[i], "w": w912, "red": red, "summ": summ, "rbm": rbm, "attb": attb}
        for i in range(8)
    ]
    trace = bool(int(os.environ.get("KERNEL_TRACE", "0")))
    if trace:
        _ensure_ntff_hook()
    res = run_bass_kernel_spmd(nc, in_maps, list(range(8)), trace=trace)
    if trace:
        print(f"HW exec time: {res.exec_time_ns} ns (mean {res.mean_exec_time_ns})")
        _CACHE["last_results"] = res

    full = np.empty((B, N, 3 * C, H, W), dtype=np.float32)
    for core in range(8):
        b, q = divmod(core, 4)
        full[b, :, :, q * HQ : (q + 1) * HQ, :] = res.results[core]["out"]
    return full


# revision 10
# speedup vs baseline: 3.6692x; 3.6692x over previous
"""Trainium2 Bass kernel for nn_PointWiseGlobalFusion (v1, bf16 compute).

Reference computation (B=2, N=5, C=64, H=W=256, G=4 groups, nf=64):
  emb1 = grouped_conv3x3(x, w1, b1); emb2 = grouped_conv3x3(x, w2, b2)
  cor[n,m,g,h,w] = sum_c emb1[n,g,c,h,w] * emb2[m,g,c,h,w]
  att  = softmax_m(cor);  nl[n] = sum_m att[n,m] * x[m]
  out  = concat([x, nl, broadcast(max_n x)], channel axis)

Sharding: 8 cores = (b in {0,1}) x (4 H-quarters of 64 rows).

Per-core kernel layout:
  xb  bf16 [128, 5, 66, 258]: rows 0-63 = padded x channels (dy=0 view),
      rows 64-127 = same channels shifted one image row down (dy=1 view).
      3x3 conv = 6 matmuls/frame: 3 dx-shifts over the dy{0,1} pair (K=128)
      + 3 dx-shifts for dy=2 (weights zero in the upper half).
  xw  bf16 [128, 5, 64, 256]: center x, channels duplicated in both halves
      (for pair-stacked weighted-sum muls and the frame-max pool).
  xpass f32 [5, 64, 64, 256]: exact passthrough, DRAM->DRAM DMA only.
  E / Eswap bf16 [128, 5f, 1024px]: (emb1|emb2) and (emb2|emb1); one
      tensor_mul of E[:,i] * Eswap[:,j] yields products for ordered pairs
      (i,j) AND (j,i) stacked in partitions -> 15 muls cover all 25 pairs.
  cor, softmax, att-broadcast, weighted-sum accumulation via PE matmuls
      with host-built mask/identity lhsT tensors; nl accumulates in PSUM
      (fp32) and is DMA'd straight to the output.
"""

import os
import sys

import numpy as np

if "/opt/trn_rl_repo" not in sys.path:
    sys.path.insert(0, "/opt/trn_rl_repo")

import ml_dtypes

BF16 = ml_dtypes.bfloat16

B, N, C, H, W = 2, 5, 64, 256, 256
G, CG = 4, 16
HQ = H // 4  # 64 rows per core
NPAIR = N * N

RS = 4  # rows per strip
NSTRIP = HQ // RS
RC = 2  # rows per psum chunk
CPX = RC * W  # 512
SPX = RS * W  # 1024

UPAIRS = [(i, j) for i in range(N) for j in range(i, N)]  # 15 unordered
NQ = [(0, 1), (2, 3), (4, None)]  # n-pair stacks for the weighted sum

_CACHE = {}


def _masks():
    # cor reduce: lhsT[u] maps product-stack rows -> cor rows 4*(5n+m)+g
    red2 = np.zeros((len(UPAIRS), 128, 4 * NPAIR), dtype=np.float32)
    for u, (i, j) in enumerate(UPAIRS):
        for c in range(C):
            red2[u, c, 4 * (5 * i + j) + c // CG] = 1.0
            if i != j:
                red2[u, C + c, 4 * (5 * j + i) + c // CG] = 1.0
    # softmax denominator: cor row 4*(5n+m)+g -> col 4n+g
    summ = np.zeros((4 * NPAIR, 4 * N), dtype=np.float32)
    # recip broadcast back: row 4n+g -> cols 4*(5n+m)+g
    rbm = np.zeros((4 * N, 4 * NPAIR), dtype=np.float32)
    for n in range(N):
        for m in range(N):
            for g in range(G):
                summ[4 * (5 * n + m) + g, 4 * n + g] = 1.0
                rbm[4 * n + g, 4 * (5 * n + m) + g] = 1.0
    # att broadcast over c for the stacked ws: stack t=(q,m):
    #   cols 0-63 <- att row 4*(5a+m)+g ; cols 64-127 <- att row 4*(5b+m)+g
    attb = np.zeros((3 * N, 4 * NPAIR, 128), dtype=np.float32)
    for q, (a, b) in enumerate(NQ):
        for m in range(N):
            t = q * N + m
            for g in range(G):
                attb[t, 4 * (5 * a + m) + g, g * CG : (g + 1) * CG] = 1.0
                if b is not None:
                    attb[t, 4 * (5 * b + m) + g, C + g * CG : C + (g + 1) * CG] = 1.0
    ident = np.eye(128, dtype=np.float32)
    return red2, summ, rbm, attb, ident


def _conv_weights(w1, b1, w2, b2):
    """wt[6, 128, 128] bf16 lhsT: slots 0-2 dx with dy{0,1}; 3-5 dx with dy2."""
    wt = np.zeros((6, 128, 2 * C), dtype=np.float32)
    for dx in range(3):
        for o in range(C):
            g = o // CG
            r = slice(g * CG, (g + 1) * CG)
            wt[dx, r, o] = w1[o, :, 0, dx]
            wt[dx, r, C + o] = w2[o, :, 0, dx]
            r2 = slice(C + g * CG, C + (g + 1) * CG)
            wt[dx, r2, o] = w1[o, :, 1, dx]
            wt[dx, r2, C + o] = w2[o, :, 1, dx]
            wt[3 + dx, r, o] = w1[o, :, 2, dx]
            wt[3 + dx, r, C + o] = w2[o, :, 2, dx]
    b12 = np.concatenate([b1, b2]).astype(np.float32).reshape(128, 1)
    return wt, b12


def _build_bass():
    import concourse.bacc as bacc
    import concourse.mybir as mybir
    from concourse import tile

    F32 = mybir.dt.float32
    BF = mybir.dt.bfloat16
    IDN = mybir.ActivationFunctionType.Identity
    EXP = mybir.ActivationFunctionType.Exp
    nc = bacc.Bacc("TRN2", target_bir_lowering=False)

    xb_in = nc.dram_tensor("xb", [128, N, HQ + 2, W + 2], BF, kind="ExternalInput")
    xw_in = nc.dram_tensor("xw", [128, N, HQ, W], BF, kind="ExternalInput")
    xp_in = nc.dram_tensor("xpass", [N, C, HQ, W], F32, kind="ExternalInput")
    wt_in = nc.dram_tensor("wt", [6, 128, 128], BF, kind="ExternalInput")
    b12_in = nc.dram_tensor("b12", [128, 1], F32, kind="ExternalInput")
    red_in = nc.dram_tensor("red2", [15, 128, 4 * NPAIR], BF, kind="ExternalInput")
    summ_in = nc.dram_tensor("summ", [4 * NPAIR, 4 * N], BF, kind="ExternalInput")
    rbm_in = nc.dram_tensor("rbm", [4 * N, 4 * NPAIR], F32, kind="ExternalInput")
    attb_in = nc.dram_tensor("attb", [15, 4 * NPAIR, 128], BF, kind="ExternalInput")
    id_in = nc.dram_tensor("ident", [128, 128], BF, kind="ExternalInput")
    out = nc.dram_tensor("out", [N, 3 * C, HQ, W], F32, kind="ExternalOutput")

    with tile.TileContext(nc) as tc:
        with (
            tc.tile_pool(name="const", bufs=1) as cp,
            tc.tile_pool(name="xsb", bufs=2) as xp,
            tc.tile_pool(name="emb", bufs=2) as ep,
            tc.tile_pool(name="prod", bufs=1) as prp,
            tc.tile_pool(name="small", bufs=3) as sp,
            tc.tile_pool(name="psE", bufs=1, space="PSUM") as psE,
            tc.tile_pool(name="psC", bufs=2, space="PSUM") as psC,
            tc.tile_pool(name="psS", bufs=1, space="PSUM") as psS,
            tc.tile_pool(name="psR", bufs=1, space="PSUM") as psR,
            tc.tile_pool(name="psA", bufs=1, space="PSUM") as psA,
            tc.tile_pool(name="psN", bufs=2, space="PSUM") as psN,
        ):
            wt_t = cp.tile([128, 6, 128], BF)
            nc.sync.dma_start(wt_t[:], wt_in[:].transpose([1, 0, 2]))
            b12_t = cp.tile([128, 1], F32)
            nc.sync.dma_start(b12_t[:], b12_in[:])
            red_t = cp.tile([128, 15, 4 * NPAIR], BF)
            nc.sync.dma_start(red_t[:], red_in[:].transpose([1, 0, 2]))
            summ_t = cp.tile([4 * NPAIR, 4 * N], BF)
            nc.sync.dma_start(summ_t[:], summ_in[:])
            rbm_t = cp.tile([4 * N, 4 * NPAIR], F32)
            nc.sync.dma_start(rbm_t[:], rbm_in[:])
            attb_t = cp.tile([4 * NPAIR, 15, 128], BF)
            nc.sync.dma_start(attb_t[:], attb_in[:].transpose([1, 0, 2]))
            id_t = cp.tile([128, 128], BF)
            nc.sync.dma_start(id_t[:], id_in[:])

            # exact passthrough: DRAM -> DRAM
            for f in range(N):
                nc.sync.dma_start(out[f, 0:C, :, :], xp_in[f, :, :, :])

            for s in range(NSTRIP):
                r0 = s * RS
                xbs = xp.tile([128, N, RS + 2, W + 2], BF, tag="xbs")
                nc.sync.dma_start(xbs[:], xb_in[:, :, r0 : r0 + RS + 2, :])
                xws = xp.tile([128, N, RS, W], BF, tag="xws")
                nc.sync.dma_start(xws[:], xw_in[:, :, r0 : r0 + RS, :])

                # frame-max pool (bf16 tree, last step casts to f32)
                pa = sp.tile([C, SPX], BF, tag="poolA")
                pb = sp.tile([C, SPX], BF, tag="poolB")
                pf = sp.tile([C, SPX], F32, tag="poolF")
                nc.vector.tensor_max(pa[:], xws[0:C, 0, :, :], xws[0:C, 1, :, :])
                nc.vector.tensor_max(pb[:], xws[0:C, 2, :, :], xws[0:C, 3, :, :])
                nc.vector.tensor_max(pa[:], pa[:], pb[:])
                nc.vector.tensor_max(pf[:], pa[:], xws[0:C, 4, :, :])
                for f in range(N):
                    nc.sync.dma_start(out[f, 2 * C : 3 * C, r0 : r0 + RS, :], pf[:])

                # ---- both grouped convs, all 5 frames, 4 rows ----
                E = ep.tile([128, N, SPX], BF, tag="E")
                Es = ep.tile([128, N, SPX], BF, tag="Es")
                for f in range(N):
                    for rc2 in range(RS // RC):
                        pe = psE.tile([128, CPX], F32)
                        for j in range(6):
                            dy0 = 0 if j < 3 else 2
                            dx = j % 3
                            rhs = xbs[:, f, rc2 * RC + dy0 : rc2 * RC + dy0 + RC, dx : dx + W]
                            nc.tensor.matmul(
                                pe[:], wt_t[:, j, :], rhs, start=(j == 0), stop=(j == 5)
                            )
                        sl = slice(rc2 * CPX, (rc2 + 1) * CPX)
                        nc.scalar.activation(E[:, f, sl], pe[:], IDN, bias=b12_t[:, 0:1])
                    # swapped copy (emb2|emb1) for pair-stacked products
                    nc.vector.tensor_copy(Es[0:C, f, :], E[C : 2 * C, f, :])
                    nc.vector.tensor_copy(Es[C : 2 * C, f, :], E[0:C, f, :])

                # ---- correlation products (both chunks at once) ----
                prods = prp.tile([128, 15, SPX], BF, tag="prods")
                for u, (i, j) in enumerate(UPAIRS):
                    nc.vector.tensor_mul(prods[:, u, :], E[:, i, :], Es[:, j, :])

                for rc2 in range(RS // RC):
                    sl = slice(rc2 * CPX, (rc2 + 1) * CPX)
                    rows = slice(r0 + rc2 * RC, r0 + rc2 * RC + RC)

                    pc = psC.tile([4 * NPAIR, CPX], F32)
                    for u in range(15):
                        nc.tensor.matmul(
                            pc[:], red_t[:, u, :], prods[:, u, sl],
                            start=(u == 0), stop=(u == 14),
                        )

                    exps = sp.tile([4 * NPAIR, CPX], BF, tag="exps")
                    nc.scalar.activation(exps[:], pc[:], EXP)
                    ps_s = psS.tile([4 * N, CPX], F32)
                    nc.tensor.matmul(ps_s[:], summ_t[:], exps[:], start=True, stop=True)
                    rec = sp.tile([4 * N, CPX], F32, tag="rec")
                    nc.vector.reciprocal(rec[:], ps_s[:])
                    ps_rb = psR.tile([4 * NPAIR, CPX], F32)
                    nc.tensor.matmul(ps_rb[:], rbm_t[:], rec[:], start=True, stop=True)
                    att = sp.tile([4 * NPAIR, CPX], BF, tag="att")
                    nc.vector.tensor_mul(att[:], exps[:], ps_rb[:])

                    # ---- weighted sum over frames, n-pair stacked ----
                    for q, (a, b) in enumerate(NQ):
                        pnl = psN.tile([128, CPX], F32)
                        for m in range(N):
                            t = q * N + m
                            pab = psA.tile([128, CPX], F32)
                            nc.tensor.matmul(
                                pab[:], attb_t[:, t, :], att[:], start=True, stop=True
                            )
                            pw = sp.tile([128, CPX], BF, tag="pw")
                            xm = xws[:, m, rc2 * RC : rc2 * RC + RC, :]
                            nc.vector.tensor_mul(pw[:], pab[:], xm)
                            nc.tensor.matmul(
                                pnl[:], id_t[:], pw[:], start=(m == 0), stop=(m == N - 1)
                            )
                        nlf = sp.tile([128, CPX], F32, tag="nlf")
                        nc.scalar.copy(nlf[:], pnl[:])
                        nc.sync.dma_start(out[a, C : 2 * C, rows, :], nlf[0:C, :])
                        if b is not None:
                            nc.sync.dma_start(out[b, C : 2 * C, rows, :], nlf[C : 2 * C, :])

    nc.compile()
    return nc


def _get_nc():
    if "nc" not in _CACHE:
        _CACHE["nc"] = _build_bass()
    return _CACHE["nc"]


def _shard_x(x):
    """-> per-core dicts with xb (bf16 dy-stacked), xw (bf16 dup), xpass f32."""
    xpad = np.pad(x, ((0, 0), (0, 0), (0, 0), (1, 1), (1, 1)))
    shards = []
    for core in range(8):
        b, q = divmod(core, 4)
        sl = xpad[b, :, :, q * HQ : q * HQ + HQ + 2, :]  # [5, 64, 66, 258] f32
        slt = sl.transpose(1, 0, 2, 3)  # [64, 5, 66, 258]
        xb = np.zeros((128, N, HQ + 2, W + 2), dtype=BF16)
        xb[0:C] = slt.astype(BF16)
        xb[C:, :, 0 : HQ + 1, :] = slt[:, :, 1:, :].astype(BF16)
        xc = x[b, :, :, q * HQ : (q + 1) * HQ, :]  # [5, 64, 64, 256] f32
        xct = xc.transpose(1, 0, 2, 3).astype(BF16)
        xw = np.concatenate([xct, xct], axis=0)  # [128, 5, 64, 256]
        shards.append(
            {
                "xb": np.ascontiguousarray(xb),
                "xw": np.ascontiguousarray(xw),
                "xpass": np.ascontiguousarray(xc.astype(np.float32)),
            }
        )
    return shards


def _ensure_ntff_hook():
    import types

    try:
        from antenv.axon_hooks import get_axon_ntff_profile_hook  # noqa: F401

        return
    except ImportError:
        pass
    import antenv

    mod = types.ModuleType("antenv.axon_hooks")
    _state = {"hook": None}
    mod.set_axon_ntff_profile_hook = lambda h: _state.__setitem__("hook", h)
    mod.get_axon_ntff_profile_hook = lambda: _state["hook"]
    sys.modules["antenv.axon_hooks"] = mod
    antenv.axon_hooks = mod
    try:
        from trn_agent_boot.trn_boot import _ntff_profile_via_ctypes

        mod.set_axon_ntff_profile_hook(
            _ntff_profile_via_ctypes("/opt/axon/libaxon_pjrt.so")
        )
    except Exception as e:
        print(f"ntff hook setup failed: {e}", file=sys.stderr)


def kernel(x, w1, b1, w2, b2):
    from concourse.bass_utils import run_bass_kernel_spmd

    x = np.asarray(x, dtype=np.float32)
    nc = _get_nc()
    wt, b12 = _conv_weights(
        np.asarray(w1, np.float32), np.asarray(b1, np.float32),
        np.asarray(w2, np.float32), np.asarray(b2, np.float32),
    )
    red2, summ, rbm, attb, ident = _masks()
    consts = {
        "wt": wt.astype(BF16), "b12": b12,
        "red2": red2.astype(BF16), "summ": summ.astype(BF16),
        "rbm": rbm, "attb": attb.astype(BF16), "ident": ident.astype(BF16),
    }
    shards = _shard_x(x)
    in_maps = [dict(shards[i], **consts) for i in range(8)]
    trace = bool(int(os.environ.get("KERNEL_TRACE", "0")))
    if trace:
        _ensure_ntff_hook()
    res = run_bass_kernel_spmd(nc, in_maps, list(range(8)), trace=trace)
    if trace:
        print(f"HW exec time: {res.exec_time_ns} ns (mean {res.mean_exec_time_ns})")
        _CACHE["last_results"] = res

    full = np.empty((B, N, 3 * C, H, W), dtype=np.float32)
    for core in range(8):
        b, q = divmod(core, 4)
        full[b, :, :, q * HQ : (q + 1) * HQ, :] = res.results[core]["out"]
    return full
